# revision 1
# baseline (speedup 1.0000x reference)
"""Causal self-attention kernel for 8 Trainium2 NeuronCores.

Problem: B=4, S=2048, D=1024, H=16, HD=64 (fp32).
  qkv = x @ w_qkv.T ; per-head causal softmax attention ; out @ w_proj.T

Sharding: core c handles batch b = c//2 and head-half hh = c%2 (8 heads).
Each core computes its 8 heads' attention and a partial output projection
(w_proj column slice); the host sums the two partials per batch.

Device-side layout strategy ("transposed" dataflow, no on-device transposes):
  - host supplies x.T and pre-transposed weights (d-major), so Q^T/K^T land
    naturally as [head_dim, seq] and V as [seq, head_dim] from matmuls
  - scores are computed transposed S'[k, q] = K^T.T @ Q^T for a PAIR of heads
    into one 2-bank psum tile; softmax runs as one exp (ACT) over the pair ->
    causal zeroing (GPSIMD affine_select) -> AV matmuls with a ones-column
    appended to V producing sumexp in psum row 64 ->
    reciprocal + partition_broadcast + multiply
  - all matmuls run in float32r (full PE rate, ~1e-4 rel err)
"""

import sys

if "/opt/trn_rl_repo" not in sys.path:
    sys.path.insert(0, "/opt/trn_rl_repo")

import numpy as np

import concourse.tile as tile
from concourse import bacc, mybir

F32 = mybir.dt.float32
F32R = mybir.dt.float32r
EXP = mybir.ActivationFunctionType.Exp

B, S, D = 4, 2048, 1024
H, HD = 16, 64
P = 128
DT = D // P            # 8 d-tiles (contraction tiles for projections)
NHC = 8                # heads per core
NG = NHC // 2          # head pair-groups per core
QB = 4                 # q-blocks of 512
QW = 512               # q-block width
KT = S // P            # 16 k-tiles
XCH = 8                # xT DMA split chunks (along seq)
SCALE = 1.0 / np.sqrt(HD)

_NC = None


def _build(loop_reps=1):
    nc = bacc.Bacc("TRN2", target_bir_lowering=False, debug=False)

    xT = nc.dram_tensor("xT", [D, S], F32R, kind="ExternalInput")
    wqT = nc.dram_tensor("wqT", [D, 512], F32R, kind="ExternalInput")
    wkT = nc.dram_tensor("wkT", [D, 512], F32R, kind="ExternalInput")
    wvT = nc.dram_tensor("wvT", [D, 512], F32R, kind="ExternalInput")
    wpT = nc.dram_tensor("wpT", [512, D], F32R, kind="ExternalInput")
    y = nc.dram_tensor("y", [S, D], F32R, kind="ExternalOutput")

    with tile.TileContext(nc) as tc:
        if loop_reps > 1:
            with tc.For_i(0, loop_reps, 1):
                _body(nc, tc, xT, wqT, wkT, wvT, wpT, y)
        else:
            _body(nc, tc, xT, wqT, wkT, wvT, wpT, y)
    nc.compile()
    return nc


def _body(nc, tc, xT, wqT, wkT, wvT, wpT, y):
    with (
        tc.tile_pool(name="big", bufs=1) as big,
        tc.tile_pool(name="wsl", bufs=1) as wsl,
        tc.tile_pool(name="qk", bufs=1) as qkp,
        tc.tile_pool(name="pfull", bufs=2) as pfp,
        tc.tile_pool(name="pband", bufs=1) as pbp,
        tc.tile_pool(name="small", bufs=2) as sp,
        tc.tile_pool(name="ost", bufs=1) as ostp,
        tc.tile_pool(name="psA", bufs=3, space="PSUM") as psA,
        tc.tile_pool(name="psO", bufs=2, space="PSUM") as psO,
    ):
        # ---- persistent loads -------------------------------------------
        # xT split into seq-chunks so compute can start before the full 8MB
        xT_sb = big.tile([P, DT, S], F32R, tag="xT")
        xT_src = xT.ap().rearrange("(o p) s -> p o s", p=P)
        xw = S // XCH
        for c in range(XCH):
            nc.sync.dma_start(
                xT_sb[:, :, c * xw:(c + 1) * xw], xT_src[:, :, c * xw:(c + 1) * xw])

        wvT_sb = big.tile([P, DT, 512], F32R, tag="oall")  # slot reused by oall
        nc.sync.dma_start(wvT_sb, wvT.ap().rearrange("(o p) e -> p o e", p=P))

        wpT_sb = big.tile([P, 4, D], F32R, tag="wpT")
        nc.sync.dma_start(wpT_sb, wpT.ap().rearrange("(t p) e -> p t e", p=P))

        # V with a ones column per head: [P, kt, 8 heads * 65]
        vaug = big.tile([P, KT, NHC * 65], F32R, tag="vaug")
        ones_cols = vaug.rearrange("p t (h c) -> p t h c", c=65)[:, :, :, 64]
        nc.gpsimd.memset(ones_cols.bitcast(F32), 1.0)

        # ---- V projection (all 8 heads at once, two s-tiles per psum) ----
        for sp2 in range(KT // 2):
            pv = psA.tile([P, 2, QW], F32, tag="mm", name=f"pv_{sp2}")
            for half in range(2):
                st = 2 * sp2 + half
                for dk in range(DT):
                    nc.tensor.matmul(
                        pv[:, half, :],
                        lhsT=xT_sb[:, dk, st * P:(st + 1) * P],
                        rhs=wvT_sb[:, dk, :],
                        start=(dk == 0), stop=(dk == DT - 1),
                    )
            nc.vector.tensor_copy(
                out=vaug[:, 2 * sp2:2 * sp2 + 2, :]
                    .rearrange("p t (h c) -> p t h c", c=65)[:, :, :, 0:64],
                in_=pv.rearrange("p t (h c) -> p t h c", c=64),
            )

        # output accumulator O'[do, q] (do = local_head*64 + hd), normalized
        oall = big.tile([P, NG, S], F32R, tag="oall")

        # ---- per head-pair-group: Q/K projection + attention ------------
        for g in range(NG):
            wqTg = wsl.tile([P, DT, P], F32R, tag="wq", name=f"wq_{g}")
            nc.sync.dma_start(
                wqTg,
                wqT.ap().rearrange("(o p) e -> p o e", p=P)[:, :, g * P:(g + 1) * P],
            )
            wkTg = wsl.tile([P, DT, P], F32R, tag="wk", name=f"wk_{g}")
            nc.sync.dma_start(
                wkTg,
                wkT.ap().rearrange("(o p) e -> p o e", p=P)[:, :, g * P:(g + 1) * P],
            )

            qTg = qkp.tile([P, S], F32R, tag="qT", name=f"qT_{g}")
            kTg = qkp.tile([P, S], F32R, tag="kT", name=f"kT_{g}")
            for w_sb, dst in ((wqTg, qTg), (wkTg, kTg)):
                for sb2 in range(2):  # two 512-blocks per psum tile
                    pq = psA.tile([P, 2, QW], F32, tag="mm",
                                  name=f"pq_{g}_{sb2}")
                    for half in range(2):
                        sb = 2 * sb2 + half
                        for dk in range(DT):
                            nc.tensor.matmul(
                                pq[:, half, :],
                                lhsT=w_sb[:, dk, :],
                                rhs=xT_sb[:, dk, sb * QW:(sb + 1) * QW],
                                start=(dk == 0), stop=(dk == DT - 1),
                            )
                    nc.vector.tensor_copy(
                        out=dst[:, sb2 * 2 * QW:(sb2 + 1) * 2 * QW],
                        in_=pq.rearrange("p t q -> p (t q)"),
                    )

            # ---- attention for the two heads of this group --------------
            for qb in range(QB):
                nkt = 4 * qb + 4  # causal: k-tiles 0 .. 4qb+3
                po = [
                    psO.tile([65, QW], F32, tag="po", name=f"po_{g}_{qb}_{hl}")
                    for hl in range(2)
                ]
                pband = pbp.tile([P, 2, 4, QW], F32R, tag="pband",
                                 name=f"pband_{g}_{qb}")

                for kt in range(nkt):
                    rel = kt - 4 * qb
                    v0 = 0  # bisect: no valid-col restriction
                    ps2 = psA.tile([P, 2, QW], F32, tag="mm",
                                   name=f"ps_{g}_{qb}_{kt}")
                    for hl in range(2):
                        hp = hl * 64
                        nc.tensor.matmul(
                            ps2[:, hl, v0:],
                            lhsT=kTg[hp:hp + 64, kt * P:(kt + 1) * P],
                            rhs=qTg[hp:hp + 64, qb * QW + v0:(qb + 1) * QW],
                            start=True, stop=True,
                        )
                    if rel < 0:
                        pp = pfp.tile([P, 2, QW], F32R, tag="pf",
                                      name=f"pf_{g}_{qb}_{kt}")
                        nc.scalar.activation(pp, ps2, EXP, scale=SCALE)
                        for hl in range(2):
                            h = 2 * g + hl
                            nc.tensor.matmul(
                                po[hl],
                                lhsT=vaug[:, kt, h * 65:(h + 1) * 65],
                                rhs=pp[:, hl, :],
                                start=(kt == 0), stop=False,
                            )
                    else:
                        nc.scalar.activation(
                            pband[:, :, rel, v0:], ps2[:, :, v0:], EXP,
                            scale=SCALE)

                # zero the causally-invalid region of the diagonal band:
                # keep where  q_col - 128*rel - partition >= 0
                nc.gpsimd.affine_select(
                    out=pband, in_=pband,
                    compare_op=mybir.AluOpType.is_ge, fill=0.0,
                    base=0, channel_multiplier=-1,
                    pattern=[[0, 2], [-P, 4], [1, QW]],
                )

                for rel in range(4):
                    kt = 4 * qb + rel
                    v0 = 0
                    for hl in range(2):
                        h = 2 * g + hl
                        nc.tensor.matmul(
                            po[hl][:, v0:],
                            lhsT=vaug[:, kt, h * 65:(h + 1) * 65],
                            rhs=pband[:, hl, rel, v0:],
                            start=(kt == 0), stop=(kt == nkt - 1),
                        )

                for hl in range(2):
                    recip = sp.tile([1, QW], F32, tag="recip",
                                    name=f"rc_{g}_{qb}_{hl}")
                    nc.vector.reciprocal(recip, po[hl][64:65, :])
                    bc = sp.tile([64, QW], F32, tag="bc",
                                 name=f"bc_{g}_{qb}_{hl}")
                    nc.gpsimd.partition_broadcast(bc, recip)
                    nc.vector.tensor_mul(
                        out=oall[hl * 64:(hl + 1) * 64, g, qb * QW:(qb + 1) * QW],
                        in0=po[hl][0:64, :],
                        in1=bc,
                    )

        # ---- output projection: y[s, e] = sum_do O'[do, s] * wpT[do, e] --
        for st in range(S // P):
            pf = psA.tile([P, 2, QW], F32, tag="mm", name=f"pfin_{st}")
            for eb in range(2):
                for t in range(4):
                    nc.tensor.matmul(
                        pf[:, eb, :],
                        lhsT=oall[:, t, st * P:(st + 1) * P],
                        rhs=wpT_sb[:, t, eb * QW:(eb + 1) * QW],
                        start=(t == 0), stop=(t == 3),
                    )
            ot = ostp.tile([P, D], F32R, tag="ot", name=f"ot_{st}")
            nc.vector.tensor_copy(out=ot, in_=pf.rearrange("p t q -> p (t q)"))
            nc.sync.dma_start(y.ap()[st * P:(st + 1) * P, :], ot)


def _get_nc():
    global _NC
    if _NC is None:
        _NC = _build()
    return _NC


def kernel(x, w_qkv, w_proj):
    from concourse.bass_utils import run_bass_kernel_spmd

    x = np.asarray(x, dtype=np.float32)
    w_qkv = np.asarray(w_qkv, dtype=np.float32)
    w_proj = np.asarray(w_proj, dtype=np.float32)

    in_maps = []
    for c in range(8):
        b, hh = c // 2, c % 2
        lo, hi = hh * 512, (hh + 1) * 512
        in_maps.append({
            "xT": np.ascontiguousarray(x[b].T),
            "wqT": np.ascontiguousarray(w_qkv[lo:hi].T),
            "wkT": np.ascontiguousarray(w_qkv[D + lo:D + hi].T),
            "wvT": np.ascontiguousarray(w_qkv[2 * D + lo:2 * D + hi].T),
            "wpT": np.ascontiguousarray(w_proj[:, lo:hi].T),
        })

    res = run_bass_kernel_spmd(_get_nc(), in_maps, core_ids=list(range(8)))
    out = np.empty((B, S, D), dtype=np.float32)
    for b in range(B):
        out[b] = res.results[2 * b]["y"] + res.results[2 * b + 1]["y"]
    return out

